# revision 49
# baseline (speedup 1.0000x reference)
"""Causal attention kernel for TRN2, 8 NeuronCores.

Problem: B=4, T=2048, d_in=d_out=1024 fp32 causal attention
    q = x @ Wq; k = x @ Wk; v = x @ Wv
    out = softmax(mask(q k^T)/sqrt(d)) @ v

Sharding: 2 cores per batch element. Core h of a pair owns the interleaved
query tiles {h, h+2, ..., h+14} (8 tiles of 128 rows), which balances causal
work exactly; both cores use all 2048 keys of their batch. Every core runs an
identical instruction stream (SPMD); causality and the h-offset are handled
by per-core input data (a [128,256] additive mask and the query-tile gather).

Algebraic folds (pure weight-fusion, done at "weight load time"):
  S   = q k^T   = xq (Wq Wk^T) x^T    -> M = Wq Wk^T computed once (d x d),
                                          replacing the 2048-row K projection
                                          with the 1024-row A = xq M stage
  out = P v     = (P x) Wv            -> replaces the 2048-row V projection
                                          with the per-slot B = P x stage
Per-core PE work: M 3072u + A 3072u + S 3456u + B 1152u + out 1024u
(u = 2^20 MAC) = 11776u ~= 314 us at the fp16 roofline; the scheduled
kernel simulates at ~334 us/core (95% PE busy).

Numerics: all matmuls run on the PE in fp16 at full rate (1 cycle/row).
fp32 operands are split as v = a + b with a=fp16(v), b=fp16(v-a), capturing
~22 mantissa bits. Logit-path products use 3 passes (a*a + a*b + b*a) with
fp32 PSUM accumulation, giving fp32-grade logits (the softmax here is
near-argmax with logit std ~1000, so logit precision is what matters). The
B/out stages and the softmax weights P are single-pass fp16; their error is
far below the output scale. Verified vs fp64 on CPU: max err ~0.05 vs the
fp32 reference's own ~0.08 envelope; measured 0.099 absolute (5.6e-4 of
absmax) vs the fp32 reference on HW.

Layout: the host supplies x^T, xq^T, Wq^T, Wk^T (fp16 hi/lo splits) plus x
natural, so every matmul operand is already in its natural PE layout; the
only on-chip transposes are batched 128x128 fp16 xbar DMA transposes of A,
P, and B. The S/B/out stages are software-pipelined across query slots with
all of PSUM's 8 banks partitioned as S(4) + A(2) + B/out(2).
"""

import sys
import numpy as np

for _p in (
    "/root/.axon_site",
    "/root/.axon_site/_ro/trn_rl_repo",
    "/root/.axon_site/_ro/pypackages",
    "/opt/trn_rl_repo",
):
    if _p not in sys.path:
        sys.path.append(_p)

B, T, D = 4, 2048, 1024
NQ = 8          # query tile slots per core
NKT = 16        # key tiles per batch
DC = 8          # 128-wide chunks of D
NCORES = 8

_NC = None
_PHASE_MARKS = []


def _build_nc():
    import concourse.bass as bass
    import concourse.tile as tile
    from concourse import bacc, mybir
    from contextlib import ExitStack

    f16 = mybir.dt.float16
    f32 = mybir.dt.float32
    Exp = mybir.ActivationFunctionType.Exp
    AX = mybir.AxisListType.X

    nc = bacc.Bacc("TRN2", target_bir_lowering=False, debug=False)

    xaT_d = nc.dram_tensor("xaT", [D, T], f16, kind="ExternalInput").ap()
    xbT_d = nc.dram_tensor("xbT", [D, T], f16, kind="ExternalInput").ap()
    xan_d = nc.dram_tensor("xan", [T, D], f16, kind="ExternalInput").ap()
    xqaT_d = nc.dram_tensor("xqaT", [D, NQ * 128], f16, kind="ExternalInput").ap()
    xqbT_d = nc.dram_tensor("xqbT", [D, NQ * 128], f16, kind="ExternalInput").ap()
    WqTa_d = nc.dram_tensor("WqTa", [D, D], f16, kind="ExternalInput").ap()
    WqTb_d = nc.dram_tensor("WqTb", [D, D], f16, kind="ExternalInput").ap()
    WkTa_d = nc.dram_tensor("WkTa", [D, D], f16, kind="ExternalInput").ap()
    WkTb_d = nc.dram_tensor("WkTb", [D, D], f16, kind="ExternalInput").ap()
    Wva_d = nc.dram_tensor("Wva", [D, D], f16, kind="ExternalInput").ap()
    mask_d = nc.dram_tensor("mask", [128, 256], f32, kind="ExternalInput").ap()
    out_d = nc.dram_tensor("out", [NQ, 128, D], f32, kind="ExternalOutput").ap()

    def chunked(ap):  # [D, N] dram -> [128, DC, N] (partition, d-chunk, col)
        return ap.rearrange("(c p) n -> p c n", p=128)

    with tile.TileContext(nc) as tc, ExitStack() as ctx:
        const_pool = ctx.enter_context(tc.tile_pool(name="const", bufs=1))
        mask_sb = const_pool.tile([128, 256], f32)

        xres = ctx.enter_context(tc.tile_pool(name="xres", bufs=1))
        xaT = xres.tile([128, DC, T], f16)
        xbT = xres.tile([128, DC, T], f16)

        mres = ctx.enter_context(tc.tile_pool(name="mres", bufs=1))
        Mh = mres.tile([128, DC, D], f16)   # [i-part, i-chunk, i2]
        Ml = mres.tile([128, DC, D], f16)

        wv = ctx.enter_context(tc.tile_pool(name="wv", bufs=1))
        Wva_sb = wv.tile([128, DC, D], f16)

        xqs = ctx.enter_context(tc.tile_pool(name="xqs", bufs=4))
        xq_tiles = []

        _PHASE_MARKS.append(('M', nc.next_id()))
        # ---------------- M = Wq Wk^T  (3-pass fp16, contraction over d_out)
        with ExitStack() as ph:
            wk = ph.enter_context(tc.tile_pool(name="wk", bufs=2))
            wq = ph.enter_context(tc.tile_pool(name="wq", bufs=8))
            # M-phase-critical loads go first, split across both HWDGE rings
            # (a-halves on SP, b-halves on ACT) so the first matmul starts
            # after ~1.3 MB per ring.
            wk_tiles = []
            for g in range(2):
                wka = wk.tile([128, DC, 512], f16, tag="wka", name=f"wka_{g}")
                wkb = wk.tile([128, DC, 512], f16, tag="wkb", name=f"wkb_{g}")
                wk_tiles.append((wka, wkb))
            # first-needed slices first so the first matmul can start after
            # ~2 small DMAs per ring
            wka0, wkb0 = wk_tiles[0]
            wq_tiles = []
            for m in range(DC):
                wqa = wq.tile([128, DC, 128], f16, tag="wqa", name=f"wqa_{m}")
                wqb = wq.tile([128, DC, 128], f16, tag="wqb", name=f"wqb_{m}")
                wq_tiles.append((wqa, wqb))
            nc.sync.dma_start(out=wka0[:, 0:2, :], in_=chunked(WkTa_d)[:, 0:2, 0:512])
            nc.scalar.dma_start(out=wkb0[:, 0:2, :], in_=chunked(WkTb_d)[:, 0:2, 0:512])
            nc.sync.dma_start(out=wq_tiles[0][0], in_=chunked(WqTa_d)[:, :, 0:128])
            nc.scalar.dma_start(out=wq_tiles[0][1], in_=chunked(WqTb_d)[:, :, 0:128])
            for c4 in range(1, 4):
                cs = slice(c4 * 2, c4 * 2 + 2)
                nc.sync.dma_start(out=wka0[:, cs, :], in_=chunked(WkTa_d)[:, cs, 0:512])
                nc.scalar.dma_start(out=wkb0[:, cs, :], in_=chunked(WkTb_d)[:, cs, 0:512])
            for m in range(1, DC):
                wqa, wqb = wq_tiles[m]
                nc.sync.dma_start(out=wqa, in_=chunked(WqTa_d)[:, :, m * 128 : (m + 1) * 128])
                nc.scalar.dma_start(out=wqb, in_=chunked(WqTb_d)[:, :, m * 128 : (m + 1) * 128])
            nc.sync.dma_start(out=wk_tiles[1][0], in_=chunked(WkTa_d)[:, :, 512:1024])
            nc.scalar.dma_start(out=wk_tiles[1][1], in_=chunked(WkTb_d)[:, :, 512:1024])
            # bulk loads needed from the V phase onward
            nc.scalar.dma_start(out=xaT, in_=chunked(xaT_d))
            nc.sync.dma_start(out=xbT, in_=chunked(xbT_d))
            nc.scalar.dma_start(out=Wva_sb, in_=chunked(Wva_d))
            nc.sync.dma_start(out=mask_sb, in_=mask_d)
            # prefetch the first query-tile slices on the otherwise-idle
            # Pool ring; later slots stream inside the pipeline
            for j in range(4):
                xqa = xqs.tile([128, DC, 128], f16, tag="xqa", name=f"xqa_{j}")
                xqb = xqs.tile([128, DC, 128], f16, tag="xqb", name=f"xqb_{j}")
                nc.gpsimd.dma_start(out=xqa, in_=chunked(xqaT_d)[:, :, j * 128 : (j + 1) * 128])
                nc.gpsimd.dma_start(out=xqb, in_=chunked(xqbT_d)[:, :, j * 128 : (j + 1) * 128])
                xq_tiles.append((xqa, xqb))
            pp = ph.enter_context(tc.tile_pool(name="pp", bufs=4, space="PSUM"))
            for g in range(2):           # i2 groups of 512 (outer: halves WkT load)
                wka, wkb = wk_tiles[g]
                for m in range(DC):      # i-chunk of M's partition dim
                    wqa, wqb = wq_tiles[m]
                    ps = pp.tile([128, 512], f32, tag="pp")
                    for c in range(DC):  # contraction chunks over d_out
                        la = wqa[:, c, :]
                        lb = wqb[:, c, :]
                        ra = wka[:, c, :]
                        rb = wkb[:, c, :]
                        nc.tensor.matmul(ps, la, ra, start=(c == 0), stop=False)
                        nc.tensor.matmul(ps, la, rb, start=False, stop=False)
                        nc.tensor.matmul(ps, lb, ra, start=False, stop=(c == DC - 1))
                    da = Mh[:, m, g * 512 : (g + 1) * 512]
                    nc.vector.tensor_copy(da, ps)
                    nc.vector.tensor_sub(Ml[:, m, g * 512 : (g + 1) * 512], ps, da)

        _PHASE_MARKS.append(('V', nc.next_id()))
        # x in natural [T, D] layout (fp16 hi), for the B = P @ x stage
        vpool = ctx.enter_context(tc.tile_pool(name="vpool", bufs=1))
        xan = vpool.tile([128, NKT, D], f16)
        nc.scalar.dma_start(out=xan, in_=xan_d.rearrange("(kt p) i -> p kt i", p=128))

        # ---------------- attention, software-pipelined over 8 slots
        abuf = ctx.enter_context(tc.tile_pool(name="abuf", bufs=2))
        att = ctx.enter_context(tc.tile_pool(name="att", bufs=2))
        ptp = ctx.enter_context(tc.tile_pool(name="ptp", bufs=1))
        stat = ctx.enter_context(tc.tile_pool(name="stat", bufs=3))
        sp = ctx.enter_context(tc.tile_pool(name="spsum", bufs=1, space="PSUM"))
        ap_ = ctx.enter_context(tc.tile_pool(name="apsum", bufs=1, space="PSUM"))
        bop = ctx.enter_context(tc.tile_pool(name="bopsum", bufs=1, space="PSUM"))

        a_state = [None] * NQ
        s_state = [None] * NQ

        def emit_A(j):
            _PHASE_MARKS.append((f'A{j}', nc.next_id()))
            # A[q, i2] = sum_i xq[q, i] M[i, i2]   (3-pass)
            if j < len(xq_tiles):
                xqa, xqb = xq_tiles[j]
            else:
                xqa = xqs.tile([128, DC, 128], f16, tag="xqa", name=f"xqa_{j}")
                xqb = xqs.tile([128, DC, 128], f16, tag="xqb", name=f"xqb_{j}")
                nc.gpsimd.dma_start(out=xqa, in_=chunked(xqaT_d)[:, :, j * 128 : (j + 1) * 128])
                nc.gpsimd.dma_start(out=xqb, in_=chunked(xqbT_d)[:, :, j * 128 : (j + 1) * 128])
            aps = ap_.tile([128, D], f32, tag="A", name=f"aps_{j}")
            for g in range(2):
                sl = aps[:, g * 512 : (g + 1) * 512]
                for c in range(DC):
                    la = xqa[:, c, :]
                    lb = xqb[:, c, :]
                    ra = Mh[:, c, g * 512 : (g + 1) * 512]
                    rb = Ml[:, c, g * 512 : (g + 1) * 512]
                    nc.tensor.matmul(sl, la, ra, start=(c == 0), stop=False)
                    nc.tensor.matmul(sl, la, rb, start=False, stop=False)
                    nc.tensor.matmul(sl, lb, ra, start=False, stop=(c == DC - 1))
            Ah = abuf.tile([128, D], f16, tag="Ah", name=f"ah_{j}")
            Al = abuf.tile([128, D], f16, tag="Al", name=f"al_{j}")
            nc.vector.tensor_copy(Ah, aps)
            nc.vector.tensor_sub(Al, aps, Ah)
            AhT = abuf.tile([128, DC, 128], f16, tag="AhT", name=f"aht_{j}")
            AlT = abuf.tile([128, DC, 128], f16, tag="AlT", name=f"alt_{j}")
            # batched xbar transpose: out[p, c, q] = in[q, c*128+p];
            # two rings so the pair runs in parallel
            nc.sync.dma_start_transpose(AhT, Ah)
            nc.scalar.dma_start_transpose(AlT, Al)
            a_state[j] = (AhT, AlT)

        def emit_S(j):
            _PHASE_MARKS.append((f'S{j}', nc.next_id()))
            # S[q, s] = sum_i2 A[q, i2] x[s, i2]   (3-pass) + mask + softmax
            AhT, AlT = a_state[j]
            nk = 2 * j + 2
            L = nk * 128
            s = sp.tile([128, 2048], f32, tag="S", name=f"s_{j}")
            ng = (L + 511) // 512
            for g in range(ng):
                n = min(512, L - g * 512)
                sl = s[:, g * 512 : g * 512 + n]
                for c in range(DC):
                    la = AhT[:, c, :]
                    lb = AlT[:, c, :]
                    ra = xaT[:, c, g * 512 : g * 512 + n]
                    rb = xbT[:, c, g * 512 : g * 512 + n]
                    nc.tensor.matmul(sl, la, ra, start=(c == 0), stop=False)
                    nc.tensor.matmul(sl, la, rb, start=False, stop=False)
                    nc.tensor.matmul(sl, lb, ra, start=False, stop=(c == DC - 1))
            nc.vector.tensor_add(s[:, L - 256 : L], s[:, L - 256 : L], mask_sb)
            nmx = stat.tile([128, 1], f32, tag="nmx", name=f"nmx_{j}")
            nc.vector.reduce_max(nmx, s[:, :L], axis=AX, negate=True)
            nbias = stat.tile([128, 1], f32, tag="nbias", name=f"nbias_{j}")
            nc.vector.tensor_scalar_mul(nbias, nmx, 0.03125)
            P = att.tile([128, 2048], f16, tag="P", name=f"p_{j}")
            rsum = stat.tile([128, 1], f32, tag="rsum", name=f"rsum_{j}")
            nc.scalar.activation(
                out=P[:, :L], in_=s[:, :L], func=Exp,
                bias=nbias, scale=0.03125, accum_out=rsum,
            )
            rinv = stat.tile([128, 1], f32, tag="rinv", name=f"rinv_{j}")
            nc.vector.reciprocal(rinv, rsum)
            PT = ptp.tile([128, NKT, 128], f16, tag="ptc", name=f"pt_{j}")
            nc.sync.dma_start_transpose(PT[:, :nk, :], P[:, :L])
            s_state[j] = (PT, rinv)

        def emit_B(j, pool=None):
            # B = P @ x  (fold: P V = (P x) Wv), accumulated over key chunks
            _PHASE_MARKS.append((f'B{j}', nc.next_id()))
            nk = 2 * j + 2
            PT, rinv = s_state[j]
            bps = (pool or bop).tile([128, D], f32, tag="A" if pool else "bo",
                                     name=f"bps_{j}")
            for ig in range(2):
                sl = bps[:, ig * 512 : (ig + 1) * 512]
                for kc in range(nk):
                    nc.tensor.matmul(
                        sl, PT[:, kc, :], xan[:, kc, ig * 512 : (ig + 1) * 512],
                        start=(kc == 0), stop=(kc == nk - 1),
                    )
            Bh = abuf.tile([128, D], f16, tag="Bh", name=f"bh_{j}")
            nc.vector.tensor_copy(Bh, bps)
            BT = abuf.tile([128, DC, 128], f16, tag="BT", name=f"bt_{j}")
            nc.scalar.dma_start_transpose(BT, Bh)
            s_state[j] = (BT, rinv, bps)

        def emit_out(j):
            # out = (B Wv) * rinv ; reuses B's psum banks after the cast
            _PHASE_MARKS.append((f'O{j}', nc.next_id()))
            BT, rinv, bps = s_state[j]
            out_sb = att.tile([128, D], f32, tag="osb", name=f"osb_{j}")
            for og in range(2):
                sl = bps[:, og * 512 : (og + 1) * 512]
                for c in range(DC):
                    nc.tensor.matmul(
                        sl, BT[:, c, :], Wva_sb[:, c, og * 512 : (og + 1) * 512],
                        start=(c == 0), stop=(c == DC - 1),
                    )
                nc.vector.tensor_scalar_mul(out_sb[:, og * 512 : (og + 1) * 512], sl, rinv)
            nc.scalar.dma_start(out=out_d[j], in_=out_sb)
            s_state[j] = None

        _PHASE_MARKS.append(('ATT', nc.next_id()))
        # pipeline: A runs two slots ahead of S (so the A->split->transpose
        # chain is off the PE critical path); the B and out stages of slot
        # j-1 bracket A(j+2) so the B->cast->transpose->out chain is covered
        # by A's matmuls.
        emit_A(0)
        emit_A(1)
        for j in range(NQ - 1):
            emit_S(j)
            if j >= 1:
                emit_B(j - 1)
            if j + 2 < NQ:
                emit_A(j + 2)
            if j >= 1:
                emit_out(j - 1)
        # tail: B7 borrows the (now idle) A psum banks so it can run before
        # out6, whose matmuls then cover B7's cast+transpose latency.
        emit_S(NQ - 1)
        emit_B(NQ - 2)
        emit_B(NQ - 1, pool=ap_)
        emit_out(NQ - 2)
        emit_out(NQ - 1)

    nc.compile()
    return nc


def _get_nc():
    global _NC
    if _NC is None:
        _NC = _build_nc()
    return _NC


def _prep_inputs(vector, W_queries, W_keys, W_values):
    vector = np.asarray(vector, dtype=np.float32)
    Wq = np.asarray(W_queries, dtype=np.float32)
    Wk = np.asarray(W_keys, dtype=np.float32)
    Wv = np.asarray(W_values, dtype=np.float32)

    def split16(x):
        a = x.astype(np.float16)
        b = (x - a.astype(np.float32)).astype(np.float16)
        return a, b

    xa, xb = split16(vector)                            # [B, T, D]
    xaT = np.ascontiguousarray(xa.transpose(0, 2, 1))   # [B, D, T]
    xbT = np.ascontiguousarray(xb.transpose(0, 2, 1))
    WqTa, WqTb = split16(np.ascontiguousarray(Wq.T))
    WkTa, WkTb = split16(np.ascontiguousarray(Wk.T))
    Wva = Wv.astype(np.float16)

    r = np.arange(128)[:, None]
    c2 = np.arange(256)[None, :]
    masks = [
        np.where(c2 <= h * 128 + r, np.float32(0.0), np.float32(-1e30)).astype(np.float32)
        for h in (0, 1)
    ]

    in_maps = []
    for core in range(NCORES):
        b, h = core // 2, core % 2
        xqaT = np.ascontiguousarray(
            xaT[b].reshape(D, NKT, 128)[:, h::2, :].reshape(D, NQ * 128)
        )
        xqbT = np.ascontiguousarray(
            xbT[b].reshape(D, NKT, 128)[:, h::2, :].reshape(D, NQ * 128)
        )
        in_maps.append({
            "xaT": xaT[b], "xbT": xbT[b], "xan": xa[b],
            "xqaT": xqaT, "xqbT": xqbT,
            "WqTa": WqTa, "WqTb": WqTb, "WkTa": WkTa, "WkTb": WkTb, "Wva": Wva,
            "mask": masks[h],
        })
    return in_maps


def kernel(vector, W_queries, W_keys, W_values):
    from concourse.bass_utils import run_bass_kernel_spmd

    in_maps = _prep_inputs(vector, W_queries, W_keys, W_values)
    res = run_bass_kernel_spmd(_get_nc(), in_maps, core_ids=list(range(NCORES)))
    out = np.empty((B, T, D), dtype=np.float32)
    for core in range(NCORES):
        b, h = core // 2, core % 2
        o = res.results[core]["out"]
        for j in range(NQ):
            t = 2 * j + h
            out[b, t * 128 : (t + 1) * 128, :] = o[j]
    return out


# revision 54
# speedup vs baseline: 1.0074x; 1.0074x over previous
"""Causal attention kernel for TRN2, 8 NeuronCores.

Problem: B=4, T=2048, d_in=d_out=1024 fp32 causal attention
    q = x @ Wq; k = x @ Wk; v = x @ Wv
    out = softmax(mask(q k^T)/sqrt(d)) @ v

Sharding: 2 cores per batch element. Core h of a pair owns the interleaved
query tiles {h, h+2, ..., h+14} (8 tiles of 128 rows), which balances causal
work exactly; both cores use all 2048 keys of their batch. Every core runs an
identical instruction stream (SPMD); causality and the h-offset are handled
by per-core input data (a [128,256] additive mask and the query-tile gather).

Algebraic folds (pure weight-fusion, done at "weight load time"):
  S   = q k^T   = xq (Wq Wk^T) x^T    -> M = Wq Wk^T computed once (d x d),
                                          replacing the 2048-row K projection
                                          with the 1024-row A = xq M stage
  out = P v     = (P x) Wv            -> replaces the 2048-row V projection
                                          with the per-slot B = P x stage
Per-core PE work: M 3072u + A 3072u + S 3456u + B 1152u + out 1024u
(u = 2^20 MAC) = 11776u ~= 314 us at the fp16 roofline; the scheduled
kernel simulates at ~332 us/core (96% PE busy).

Numerics: all matmuls run on the PE in fp16 at full rate (1 cycle/row).
fp32 operands are split as v = a + b with a=fp16(v), b=fp16(v-a), capturing
~22 mantissa bits. Logit-path products use 3 passes (a*a + a*b + b*a) with
fp32 PSUM accumulation, giving fp32-grade logits (the softmax here is
near-argmax with logit std ~1000, so logit precision is what matters). The
B/out stages and the softmax weights P are single-pass fp16; their error is
far below the output scale. Verified vs fp64 on CPU: max err ~0.05 vs the
fp32 reference's own ~0.08 envelope; measured 0.099 absolute (5.6e-4 of
absmax) vs the fp32 reference on HW.

Layout: the host supplies x^T, xq^T, Wq^T, Wk^T (fp16 hi/lo splits) plus x
natural, so every matmul operand is already in its natural PE layout; the
only on-chip transposes are batched 128x128 fp16 xbar DMA transposes of A,
P, and B. The S/B/out stages are software-pipelined across query slots with
all of PSUM's 8 banks partitioned as S(4) + A(2) + B/out(2).
"""

import sys
import numpy as np

for _p in (
    "/root/.axon_site",
    "/root/.axon_site/_ro/trn_rl_repo",
    "/root/.axon_site/_ro/pypackages",
    "/opt/trn_rl_repo",
):
    if _p not in sys.path:
        sys.path.append(_p)

B, T, D = 4, 2048, 1024
NQ = 8          # query tile slots per core
NKT = 16        # key tiles per batch
DC = 8          # 128-wide chunks of D
NCORES = 8

_NC = None
_PHASE_MARKS = []


def _build_nc():
    import concourse.bass as bass
    import concourse.tile as tile
    from concourse import bacc, mybir
    from contextlib import ExitStack

    f16 = mybir.dt.float16
    f32 = mybir.dt.float32
    Exp = mybir.ActivationFunctionType.Exp
    AX = mybir.AxisListType.X

    nc = bacc.Bacc("TRN2", target_bir_lowering=False, debug=False)

    xaT_d = nc.dram_tensor("xaT", [D, T], f16, kind="ExternalInput").ap()
    xbT_d = nc.dram_tensor("xbT", [D, T], f16, kind="ExternalInput").ap()
    xan_d = nc.dram_tensor("xan", [T, D], f16, kind="ExternalInput").ap()
    xqaT_d = nc.dram_tensor("xqaT", [D, NQ * 128], f16, kind="ExternalInput").ap()
    xqbT_d = nc.dram_tensor("xqbT", [D, NQ * 128], f16, kind="ExternalInput").ap()
    WqTa_d = nc.dram_tensor("WqTa", [D, D], f16, kind="ExternalInput").ap()
    WqTb_d = nc.dram_tensor("WqTb", [D, D], f16, kind="ExternalInput").ap()
    WkTa_d = nc.dram_tensor("WkTa", [D, D], f16, kind="ExternalInput").ap()
    WkTb_d = nc.dram_tensor("WkTb", [D, D], f16, kind="ExternalInput").ap()
    Wva_d = nc.dram_tensor("Wva", [D, D], f16, kind="ExternalInput").ap()
    mask_d = nc.dram_tensor("mask", [128, 256], f32, kind="ExternalInput").ap()
    out_d = nc.dram_tensor("out", [NQ, 128, D], f32, kind="ExternalOutput").ap()

    def chunked(ap):  # [D, N] dram -> [128, DC, N] (partition, d-chunk, col)
        return ap.rearrange("(c p) n -> p c n", p=128)

    with tile.TileContext(nc) as tc, ExitStack() as ctx:
        const_pool = ctx.enter_context(tc.tile_pool(name="const", bufs=1))
        mask_sb = const_pool.tile([128, 256], f32)

        xres = ctx.enter_context(tc.tile_pool(name="xres", bufs=1))
        xaT = xres.tile([128, DC, T], f16)
        xbT = xres.tile([128, DC, T], f16)

        mres = ctx.enter_context(tc.tile_pool(name="mres", bufs=1))
        Mh = mres.tile([128, DC, D], f16)   # [i-part, i-chunk, i2]
        Ml = mres.tile([128, DC, D], f16)

        wv = ctx.enter_context(tc.tile_pool(name="wv", bufs=1))
        Wva_sb = wv.tile([128, DC, D], f16)

        xqs = ctx.enter_context(tc.tile_pool(name="xqs", bufs=4))
        xq_tiles = []

        _PHASE_MARKS.append(('M', nc.next_id()))
        # ---------------- M = Wq Wk^T  (3-pass fp16, contraction over d_out)
        with ExitStack() as ph:
            wk = ph.enter_context(tc.tile_pool(name="wk", bufs=2))
            wq = ph.enter_context(tc.tile_pool(name="wq", bufs=8))
            # M-phase-critical loads go first, split across both HWDGE rings
            # (a-halves on SP, b-halves on ACT) so the first matmul starts
            # after ~1.3 MB per ring.
            wk_tiles = []
            for g in range(2):
                wka = wk.tile([128, DC, 512], f16, tag="wka", name=f"wka_{g}")
                wkb = wk.tile([128, DC, 512], f16, tag="wkb", name=f"wkb_{g}")
                wk_tiles.append((wka, wkb))
            # first-needed slices first so the first matmul can start after
            # ~2 small DMAs per ring
            wka0, wkb0 = wk_tiles[0]
            wq_tiles = []
            for m in range(DC):
                wqa = wq.tile([128, DC, 128], f16, tag="wqa", name=f"wqa_{m}")
                wqb = wq.tile([128, DC, 128], f16, tag="wqb", name=f"wqb_{m}")
                wq_tiles.append((wqa, wqb))
            nc.sync.dma_start(out=wka0[:, 0:2, :], in_=chunked(WkTa_d)[:, 0:2, 0:512])
            nc.scalar.dma_start(out=wkb0[:, 0:2, :], in_=chunked(WkTb_d)[:, 0:2, 0:512])
            nc.sync.dma_start(out=wq_tiles[0][0], in_=chunked(WqTa_d)[:, :, 0:128])
            nc.scalar.dma_start(out=wq_tiles[0][1], in_=chunked(WqTb_d)[:, :, 0:128])
            for c4 in range(1, 4):
                cs = slice(c4 * 2, c4 * 2 + 2)
                nc.sync.dma_start(out=wka0[:, cs, :], in_=chunked(WkTa_d)[:, cs, 0:512])
                nc.scalar.dma_start(out=wkb0[:, cs, :], in_=chunked(WkTb_d)[:, cs, 0:512])
            for m in range(1, DC):
                wqa, wqb = wq_tiles[m]
                nc.sync.dma_start(out=wqa, in_=chunked(WqTa_d)[:, :, m * 128 : (m + 1) * 128])
                nc.scalar.dma_start(out=wqb, in_=chunked(WqTb_d)[:, :, m * 128 : (m + 1) * 128])
            nc.sync.dma_start(out=wk_tiles[1][0], in_=chunked(WkTa_d)[:, :, 512:1024])
            nc.scalar.dma_start(out=wk_tiles[1][1], in_=chunked(WkTb_d)[:, :, 512:1024])
            # bulk loads needed from the V phase onward
            nc.scalar.dma_start(out=xaT, in_=chunked(xaT_d))
            nc.sync.dma_start(out=xbT, in_=chunked(xbT_d))
            nc.scalar.dma_start(out=Wva_sb, in_=chunked(Wva_d))
            nc.sync.dma_start(out=mask_sb, in_=mask_d)
            # prefetch the first query-tile slices on the otherwise-idle
            # Pool ring; later slots stream inside the pipeline
            for j in range(4):
                xqa = xqs.tile([128, DC, 128], f16, tag="xqa", name=f"xqa_{j}")
                xqb = xqs.tile([128, DC, 128], f16, tag="xqb", name=f"xqb_{j}")
                nc.gpsimd.dma_start(out=xqa, in_=chunked(xqaT_d)[:, :, j * 128 : (j + 1) * 128])
                nc.gpsimd.dma_start(out=xqb, in_=chunked(xqbT_d)[:, :, j * 128 : (j + 1) * 128])
                xq_tiles.append((xqa, xqb))
            pp = ph.enter_context(tc.tile_pool(name="pp", bufs=4, space="PSUM"))
            for g in range(2):           # i2 groups of 512 (outer: halves WkT load)
                wka, wkb = wk_tiles[g]
                for m in range(DC):      # i-chunk of M's partition dim
                    wqa, wqb = wq_tiles[m]
                    ps = pp.tile([128, 512], f32, tag="pp")
                    for c in range(DC):  # contraction chunks over d_out
                        la = wqa[:, c, :]
                        lb = wqb[:, c, :]
                        ra = wka[:, c, :]
                        rb = wkb[:, c, :]
                        nc.tensor.matmul(ps, la, ra, start=(c == 0), stop=False)
                        nc.tensor.matmul(ps, la, rb, start=False, stop=False)
                        nc.tensor.matmul(ps, lb, ra, start=False, stop=(c == DC - 1))
                    da = Mh[:, m, g * 512 : (g + 1) * 512]
                    nc.vector.tensor_copy(da, ps)
                    nc.vector.tensor_sub(Ml[:, m, g * 512 : (g + 1) * 512], ps, da)

        _PHASE_MARKS.append(('V', nc.next_id()))
        # x in natural [T, D] layout (fp16 hi), for the B = P @ x stage
        vpool = ctx.enter_context(tc.tile_pool(name="vpool", bufs=1))
        xan = vpool.tile([128, NKT, D], f16)
        nc.scalar.dma_start(out=xan, in_=xan_d.rearrange("(kt p) i -> p kt i", p=128))

        # ---------------- attention, software-pipelined over 8 slots
        abuf = ctx.enter_context(tc.tile_pool(name="abuf", bufs=2))
        att = ctx.enter_context(tc.tile_pool(name="att", bufs=2))
        ptp = ctx.enter_context(tc.tile_pool(name="ptp", bufs=1))
        stat = ctx.enter_context(tc.tile_pool(name="stat", bufs=3))
        sp = ctx.enter_context(tc.tile_pool(name="spsum", bufs=1, space="PSUM"))
        ap_ = ctx.enter_context(tc.tile_pool(name="apsum", bufs=1, space="PSUM"))
        bop = ctx.enter_context(tc.tile_pool(name="bopsum", bufs=1, space="PSUM"))

        a_state = [None] * NQ
        s_state = [None] * NQ

        def emit_A(j, pool=None):
            _PHASE_MARKS.append((f'A{j}', nc.next_id()))
            # A[q, i2] = sum_i xq[q, i] M[i, i2]   (3-pass)
            if j < len(xq_tiles):
                xqa, xqb = xq_tiles[j]
            else:
                xqa = xqs.tile([128, DC, 128], f16, tag="xqa", name=f"xqa_{j}")
                xqb = xqs.tile([128, DC, 128], f16, tag="xqb", name=f"xqb_{j}")
                nc.gpsimd.dma_start(out=xqa, in_=chunked(xqaT_d)[:, :, j * 128 : (j + 1) * 128])
                nc.gpsimd.dma_start(out=xqb, in_=chunked(xqbT_d)[:, :, j * 128 : (j + 1) * 128])
            aps = (pool or ap_).tile([128, D], f32, tag="bo" if pool else "A",
                                     name=f"aps_{j}")
            for g in range(2):
                sl = aps[:, g * 512 : (g + 1) * 512]
                for c in range(DC):
                    la = xqa[:, c, :]
                    lb = xqb[:, c, :]
                    ra = Mh[:, c, g * 512 : (g + 1) * 512]
                    rb = Ml[:, c, g * 512 : (g + 1) * 512]
                    nc.tensor.matmul(sl, la, ra, start=(c == 0), stop=False)
                    nc.tensor.matmul(sl, la, rb, start=False, stop=False)
                    nc.tensor.matmul(sl, lb, ra, start=False, stop=(c == DC - 1))
            Ah = abuf.tile([128, D], f16, tag="Ah", name=f"ah_{j}")
            Al = abuf.tile([128, D], f16, tag="Al", name=f"al_{j}")
            nc.vector.tensor_copy(Ah, aps)
            nc.vector.tensor_sub(Al, aps, Ah)
            AhT = abuf.tile([128, DC, 128], f16, tag="AhT", name=f"aht_{j}")
            AlT = abuf.tile([128, DC, 128], f16, tag="AlT", name=f"alt_{j}")
            # batched xbar transpose: out[p, c, q] = in[q, c*128+p];
            # two rings so the pair runs in parallel
            nc.sync.dma_start_transpose(AhT, Ah)
            nc.scalar.dma_start_transpose(AlT, Al)
            a_state[j] = (AhT, AlT)

        def emit_S(j):
            _PHASE_MARKS.append((f'S{j}', nc.next_id()))
            # S[q, s] = sum_i2 A[q, i2] x[s, i2]   (3-pass) + mask + softmax
            AhT, AlT = a_state[j]
            nk = 2 * j + 2
            L = nk * 128
            s = sp.tile([128, 2048], f32, tag="S", name=f"s_{j}")
            ng = (L + 511) // 512
            for g in range(ng):
                n = min(512, L - g * 512)
                sl = s[:, g * 512 : g * 512 + n]
                for c in range(DC):
                    la = AhT[:, c, :]
                    lb = AlT[:, c, :]
                    ra = xaT[:, c, g * 512 : g * 512 + n]
                    rb = xbT[:, c, g * 512 : g * 512 + n]
                    nc.tensor.matmul(sl, la, ra, start=(c == 0), stop=False)
                    nc.tensor.matmul(sl, la, rb, start=False, stop=False)
                    nc.tensor.matmul(sl, lb, ra, start=False, stop=(c == DC - 1))
            nc.vector.tensor_add(s[:, L - 256 : L], s[:, L - 256 : L], mask_sb)
            nmx = stat.tile([128, 1], f32, tag="nmx", name=f"nmx_{j}")
            nc.vector.reduce_max(nmx, s[:, :L], axis=AX, negate=True)
            nbias = stat.tile([128, 1], f32, tag="nbias", name=f"nbias_{j}")
            nc.vector.tensor_scalar_mul(nbias, nmx, 0.03125)
            P = att.tile([128, 2048], f16, tag="P", name=f"p_{j}")
            rsum = stat.tile([128, 1], f32, tag="rsum", name=f"rsum_{j}")
            nc.scalar.activation(
                out=P[:, :L], in_=s[:, :L], func=Exp,
                bias=nbias, scale=0.03125, accum_out=rsum,
            )
            rinv = stat.tile([128, 1], f32, tag="rinv", name=f"rinv_{j}")
            nc.vector.reciprocal(rinv, rsum)
            PT = ptp.tile([128, NKT, 128], f16, tag="ptc", name=f"pt_{j}")
            nc.sync.dma_start_transpose(PT[:, :nk, :], P[:, :L])
            s_state[j] = (PT, rinv)

        def emit_B(j, pool=None):
            # B = P @ x  (fold: P V = (P x) Wv), accumulated over key chunks
            _PHASE_MARKS.append((f'B{j}', nc.next_id()))
            nk = 2 * j + 2
            PT, rinv = s_state[j]
            bps = (pool or bop).tile([128, D], f32, tag="A" if pool else "bo",
                                     name=f"bps_{j}")
            for ig in range(2):
                sl = bps[:, ig * 512 : (ig + 1) * 512]
                for kc in range(nk):
                    nc.tensor.matmul(
                        sl, PT[:, kc, :], xan[:, kc, ig * 512 : (ig + 1) * 512],
                        start=(kc == 0), stop=(kc == nk - 1),
                    )
            Bh = abuf.tile([128, D], f16, tag="Bh", name=f"bh_{j}")
            nc.vector.tensor_copy(Bh, bps)
            BT = abuf.tile([128, DC, 128], f16, tag="BT", name=f"bt_{j}")
            nc.scalar.dma_start_transpose(BT, Bh)
            s_state[j] = (BT, rinv, bps)

        def emit_out(j):
            # out = (B Wv) * rinv ; reuses B's psum banks after the cast
            _PHASE_MARKS.append((f'O{j}', nc.next_id()))
            BT, rinv, bps = s_state[j]
            out_sb = att.tile([128, D], f32, tag="osb", name=f"osb_{j}")
            for og in range(2):
                sl = bps[:, og * 512 : (og + 1) * 512]
                for c in range(DC):
                    nc.tensor.matmul(
                        sl, BT[:, c, :], Wva_sb[:, c, og * 512 : (og + 1) * 512],
                        start=(c == 0), stop=(c == DC - 1),
                    )
                nc.vector.tensor_scalar_mul(out_sb[:, og * 512 : (og + 1) * 512], sl, rinv)
            nc.scalar.dma_start(out=out_d[j], in_=out_sb)
            s_state[j] = None

        _PHASE_MARKS.append(('ATT', nc.next_id()))
        # pipeline: A runs two slots ahead of S (so the A->split->transpose
        # chain is off the PE critical path); the B and out stages of slot
        # j-1 bracket A(j+2) so the B->cast->transpose->out chain is covered
        # by A's matmuls.
        # A1 borrows the B/out psum banks (B stages start much later) so its
        # matmuls don't serialize on A0's psum being split-read.
        emit_A(0)
        emit_A(1, pool=bop)
        for j in range(NQ - 1):
            emit_S(j)
            if j >= 1:
                emit_B(j - 1)
            if j + 2 < NQ:
                emit_A(j + 2)
            if j >= 1:
                emit_out(j - 1)
        # tail: B7 borrows the (now idle) A psum banks so it can run before
        # out6, whose matmuls then cover B7's cast+transpose latency.
        emit_S(NQ - 1)
        emit_B(NQ - 2)
        emit_B(NQ - 1, pool=ap_)
        emit_out(NQ - 2)
        emit_out(NQ - 1)

    nc.compile()
    return nc


def _get_nc():
    global _NC
    if _NC is None:
        _NC = _build_nc()
    return _NC


def _prep_inputs(vector, W_queries, W_keys, W_values):
    vector = np.asarray(vector, dtype=np.float32)
    Wq = np.asarray(W_queries, dtype=np.float32)
    Wk = np.asarray(W_keys, dtype=np.float32)
    Wv = np.asarray(W_values, dtype=np.float32)

    def split16(x):
        a = x.astype(np.float16)
        b = (x - a.astype(np.float32)).astype(np.float16)
        return a, b

    xa, xb = split16(vector)                            # [B, T, D]
    xaT = np.ascontiguousarray(xa.transpose(0, 2, 1))   # [B, D, T]
    xbT = np.ascontiguousarray(xb.transpose(0, 2, 1))
    WqTa, WqTb = split16(np.ascontiguousarray(Wq.T))
    WkTa, WkTb = split16(np.ascontiguousarray(Wk.T))
    Wva = Wv.astype(np.float16)

    r = np.arange(128)[:, None]
    c2 = np.arange(256)[None, :]
    masks = [
        np.where(c2 <= h * 128 + r, np.float32(0.0), np.float32(-1e30)).astype(np.float32)
        for h in (0, 1)
    ]

    in_maps = []
    for core in range(NCORES):
        b, h = core // 2, core % 2
        xqaT = np.ascontiguousarray(
            xaT[b].reshape(D, NKT, 128)[:, h::2, :].reshape(D, NQ * 128)
        )
        xqbT = np.ascontiguousarray(
            xbT[b].reshape(D, NKT, 128)[:, h::2, :].reshape(D, NQ * 128)
        )
        in_maps.append({
            "xaT": xaT[b], "xbT": xbT[b], "xan": xa[b],
            "xqaT": xqaT, "xqbT": xqbT,
            "WqTa": WqTa, "WqTb": WqTb, "WkTa": WkTa, "WkTb": WkTb, "Wva": Wva,
            "mask": masks[h],
        })
    return in_maps


def kernel(vector, W_queries, W_keys, W_values):
    from concourse.bass_utils import run_bass_kernel_spmd

    in_maps = _prep_inputs(vector, W_queries, W_keys, W_values)
    res = run_bass_kernel_spmd(_get_nc(), in_maps, core_ids=list(range(NCORES)))
    out = np.empty((B, T, D), dtype=np.float32)
    for core in range(NCORES):
        b, h = core // 2, core % 2
        o = res.results[core]["out"]
        for j in range(NQ):
            t = 2 * j + h
            out[b, t * 128 : (t + 1) * 128, :] = o[j]
    return out
